# revision 4
# baseline (speedup 1.0000x reference)
"""Multi-head attention (B=4, S=2048, D=1024, H=16, DK=64) on 8 Trainium2
NeuronCores.

Sharding: core c = 2*b + j handles batch b = c//2 and query rows
[j*1024, (j+1)*1024).  Each core recomputes full-S K/V for its batch
(fully local, no collectives); outputs concatenate.

Per-core layout (all "transposed" = feature-on-partition):
  X^T  [D, S]    from host (pre-transposed, bf16)
  Q^T  [D, SQ]   = Wq^T-style matmul:  lhsT=Wq[c,dh], rhs=X^T[c,q]
  K^T  [D, S]    same with Wk
  V    [S, D]    natural:              lhsT=X^T[c,s], rhs=Wv[c,dv]
                 stored per s-chunk as [128, H*(DK+1)] with a ones column
                 per head (row 64 of the PV output = softmax denominator)
  E^T  [s, q]    = exp(scores^T/8 + mask - 3), streamed per (head, s-chunk)
  ctx^T[D, SQ]   normalized via reciprocal(denominator) broadcast
  out  [SQ, D]   = lhsT=ctx^T chunk, rhs=Wo  (+bo), fp32
"""

import numpy as np
import ml_dtypes

B, S, D, H, DK = 4, 2048, 1024, 16, 64
SQ = S // 2          # query rows per core
N_CORES = 8
SH = DK + 1          # per-head V width incl. ones column
NEG_C = -3.0         # exp stabilizer; cancels exactly in normalization
BF = ml_dtypes.bfloat16


def _build():
    import concourse.mybir as mybir
    import concourse.tile as tile
    from concourse import bacc

    dt = mybir.dt
    AF = mybir.ActivationFunctionType
    nc = bacc.Bacc("TRN2", num_devices=N_CORES)

    xt = nc.declare_dram_parameter("xt", [D, S], dt.bfloat16, isOutput=False)
    xqt = nc.declare_dram_parameter("xqt", [D, SQ], dt.bfloat16, isOutput=False)
    wq = nc.declare_dram_parameter("wq", [D, D], dt.bfloat16, isOutput=False)
    wk = nc.declare_dram_parameter("wk", [D, D], dt.bfloat16, isOutput=False)
    wv = nc.declare_dram_parameter("wv", [D, D], dt.bfloat16, isOutput=False)
    wo = nc.declare_dram_parameter("wo", [D, D], dt.bfloat16, isOutput=False)
    bq = nc.declare_dram_parameter("bq", [D], dt.float32, isOutput=False)
    bk = nc.declare_dram_parameter("bk", [D], dt.float32, isOutput=False)
    bv = nc.declare_dram_parameter("bv", [D], dt.float32, isOutput=False)
    bo = nc.declare_dram_parameter("bo", [D], dt.float32, isOutput=False)
    mk = nc.declare_dram_parameter("mk", [S], dt.float32, isOutput=False)
    out = nc.declare_dram_parameter("out", [SQ, D], dt.float32, isOutput=True)

    with tile.TileContext(nc) as tc:
        with (
            tc.tile_pool(name="pers", bufs=1) as pers,
            tc.tile_pool(name="ps", bufs=2, space="PSUM") as ps,
            tc.tile_pool(name="ctxp", bufs=4, space="PSUM") as ctxp,
        ):
            # ---- persistent SBUF arrays -------------------------------
            qt_s = pers.tile([128, 8 * SQ], dt.bfloat16, tag="qt")
            kt_s = pers.tile([128, 8 * S], dt.bfloat16, tag="kt")
            v_s = pers.tile([128, 16 * H * SH], dt.bfloat16, tag="v")
            ctxt_s = pers.tile([128, 8 * SQ], dt.bfloat16, tag="ctxt")
            bqc = pers.tile([128, 8], dt.float32, tag="bqc")
            bkc = pers.tile([128, 8], dt.float32, tag="bkc")
            mkc = pers.tile([128, 16], dt.float32, tag="mkc")
            bvb = pers.tile([128, D], dt.float32, tag="bvb")
            bob = pers.tile([128, D], dt.float32, tag="bob")

            nc.sync.dma_start(out=bqc, in_=bq.rearrange("(a p) -> p a", p=128))
            nc.sync.dma_start(out=bkc, in_=bk.rearrange("(a p) -> p a", p=128))
            nc.sync.dma_start(out=mkc, in_=mk.rearrange("(a p) -> p a", p=128))

            def _bcast_src(ap):
                import concourse.bass as bass
                return bass.AP(tensor=ap.tensor, offset=ap.offset,
                               ap=[[0, 128]] + [list(p) for p in ap.ap])

            nc.gpsimd.dma_start(out=bvb, in_=_bcast_src(bv[:]))
            nc.gpsimd.dma_start(out=bob, in_=_bcast_src(bo[:]))

            # ---- phase 1: QKV projections -----------------------------
            with tc.tile_pool(name="qkvin", bufs=1) as qkvin:
                xt_s = qkvin.tile([128, 8 * S], dt.bfloat16, tag="xt")
                xqt_s = qkvin.tile([128, 8 * SQ], dt.bfloat16, tag="xqt")
                wq_s = qkvin.tile([128, 8 * D], dt.bfloat16, tag="wq")
                wk_s = qkvin.tile([128, 8 * D], dt.bfloat16, tag="wk")
                wv_s = qkvin.tile([128, 8 * D], dt.bfloat16, tag="wv")
                for c in range(8):
                    nc.sync.dma_start(
                        out=xt_s[:, c * S:(c + 1) * S],
                        in_=xt[c * 128:(c + 1) * 128, :])
                    nc.sync.dma_start(
                        out=xqt_s[:, c * SQ:(c + 1) * SQ],
                        in_=xqt[c * 128:(c + 1) * 128, :])
                    nc.sync.dma_start(
                        out=wq_s[:, c * D:(c + 1) * D],
                        in_=wq[c * 128:(c + 1) * 128, :])
                    nc.sync.dma_start(
                        out=wk_s[:, c * D:(c + 1) * D],
                        in_=wk[c * 128:(c + 1) * 128, :])
                    nc.sync.dma_start(
                        out=wv_s[:, c * D:(c + 1) * D],
                        in_=wv[c * 128:(c + 1) * 128, :])

                # Q^T
                for dh in range(8):
                    pq = ps.tile([128, 1024], dt.float32, tag="ps")
                    for c in range(8):
                        lhsT = wq_s[:, c * D + dh * 128: c * D + (dh + 1) * 128]
                        for q2 in range(2):
                            nc.tensor.matmul(
                                out=pq[:, q2 * 512:(q2 + 1) * 512],
                                lhsT=lhsT,
                                rhs=xqt_s[:, c * SQ + q2 * 512: c * SQ + (q2 + 1) * 512],
                                start=(c == 0), stop=(c == 7))
                    nc.scalar.activation(
                        out=qt_s[:, dh * SQ:(dh + 1) * SQ], in_=pq,
                        func=AF.Identity, bias=bqc[:, dh:dh + 1], scale=1.0)

                # K^T
                for dh in range(8):
                    pk = [ps.tile([128, 1024], dt.float32, tag="ps",
                                  name=f"pk{dh}_{i}") for i in range(2)]
                    for c in range(8):
                        lhsT = wk_s[:, c * D + dh * 128: c * D + (dh + 1) * 128]
                        for st in range(4):
                            nc.tensor.matmul(
                                out=pk[st // 2][:, (st % 2) * 512:(st % 2 + 1) * 512],
                                lhsT=lhsT,
                                rhs=xt_s[:, c * S + st * 512: c * S + (st + 1) * 512],
                                start=(c == 0), stop=(c == 7))
                    for hf in range(2):
                        nc.scalar.activation(
                            out=kt_s[:, dh * S + hf * 1024: dh * S + (hf + 1) * 1024],
                            in_=pk[hf], func=AF.Identity,
                            bias=bkc[:, dh:dh + 1], scale=1.0)

                # V (with ones column per head)
                for sc in range(16):
                    pv = ps.tile([128, 1024], dt.float32, tag="ps")
                    for c in range(8):
                        lhsT = xt_s[:, c * S + sc * 128: c * S + (sc + 1) * 128]
                        for dv2 in range(2):
                            nc.tensor.matmul(
                                out=pv[:, dv2 * 512:(dv2 + 1) * 512],
                                lhsT=lhsT,
                                rhs=wv_s[:, c * D + dv2 * 512: c * D + (dv2 + 1) * 512],
                                start=(c == 0), stop=(c == 7))
                    v3 = v_s[:, sc * H * SH:(sc + 1) * H * SH].rearrange(
                        "p (h e) -> p h e", e=SH)
                    nc.gpsimd.memset(v3[:, :, DK:SH], 1.0)
                    for dv2 in range(2):
                        nc.vector.tensor_add(
                            out=v3[:, dv2 * 8:(dv2 + 1) * 8, 0:DK],
                            in0=pv[:, dv2 * 512:(dv2 + 1) * 512].rearrange(
                                "p (h d) -> p h d", d=DK),
                            in1=bvb[:, dv2 * 512:(dv2 + 1) * 512].rearrange(
                                "p (h d) -> p h d", d=DK))

            # ---- phase 2: attention -----------------------------------
            with (
                tc.tile_pool(name="attin", bufs=1) as attin,
                tc.tile_pool(name="epool", bufs=4) as epool,
                tc.tile_pool(name="rpool", bufs=2) as rpool,
                tc.tile_pool(name="opool", bufs=2) as opool,
            ):
                wo_s = attin.tile([128, 8 * D], dt.bfloat16, tag="wo")
                for c in range(8):
                    nc.sync.dma_start(
                        out=wo_s[:, c * D:(c + 1) * D],
                        in_=wo[c * 128:(c + 1) * 128, :])

                for h in range(H):
                    dhh, po = h // 2, (h % 2) * 64
                    cx = [ctxp.tile([SH, 512], dt.float32, tag="cx",
                                    name=f"cx{h}_{i}") for i in range(2)]
                    for sc in range(16):
                        sp = ps.tile([128, 1024], dt.float32, tag="ps")
                        lhsT = kt_s[po:po + 64,
                                    dhh * S + sc * 128: dhh * S + (sc + 1) * 128]
                        for q2 in range(2):
                            nc.tensor.matmul(
                                out=sp[:, q2 * 512:(q2 + 1) * 512],
                                lhsT=lhsT,
                                rhs=qt_s[po:po + 64,
                                         dhh * SQ + q2 * 512: dhh * SQ + (q2 + 1) * 512],
                                start=True, stop=True)
                        e = epool.tile([128, 1024], dt.bfloat16, tag="e")
                        nc.scalar.activation(
                            out=e, in_=sp, func=AF.Exp,
                            bias=mkc[:, sc:sc + 1], scale=1.0 / np.sqrt(DK))
                        vh = v_s[:, sc * H * SH + h * SH: sc * H * SH + (h + 1) * SH]
                        for q2 in range(2):
                            nc.tensor.matmul(
                                out=cx[q2], lhsT=vh,
                                rhs=e[:, q2 * 512:(q2 + 1) * 512],
                                start=(sc == 0), stop=(sc == 15))
                    rcp = rpool.tile([1, 1024], dt.float32, tag="rcp")
                    for q2 in range(2):
                        nc.vector.reciprocal(
                            out=rcp[:, q2 * 512:(q2 + 1) * 512],
                            in_=cx[q2][DK:SH, :])
                    rb = rpool.tile([64, 1024], dt.float32, tag="rb")
                    nc.gpsimd.partition_broadcast(rb, rcp[0:1, :])
                    for q2 in range(2):
                        nc.vector.tensor_mul(
                            out=ctxt_s[po:po + 64,
                                       dhh * SQ + q2 * 512: dhh * SQ + (q2 + 1) * 512],
                            in0=cx[q2][0:DK, :],
                            in1=rb[:, q2 * 512:(q2 + 1) * 512])

                # ---- phase 3: output projection -----------------------
                for qc in range(8):
                    pO = ps.tile([128, 1024], dt.float32, tag="ps")
                    for i in range(8):
                        lhsT = ctxt_s[:, i * SQ + qc * 128: i * SQ + (qc + 1) * 128]
                        for do2 in range(2):
                            nc.tensor.matmul(
                                out=pO[:, do2 * 512:(do2 + 1) * 512],
                                lhsT=lhsT,
                                rhs=wo_s[:, i * D + do2 * 512: i * D + (do2 + 1) * 512],
                                start=(i == 0), stop=(i == 7))
                    ot = opool.tile([128, 1024], dt.float32, tag="ot")
                    nc.vector.tensor_add(out=ot, in0=pO, in1=bob)
                    nc.sync.dma_start(
                        out=out[qc * 128:(qc + 1) * 128, :], in_=ot)

    nc.compile()
    return nc


def kernel(hidden_states, attention_mask, Wq, bq, Wk, bk, Wv, bv, Wo, bo):
    from concourse.bass_utils import run_bass_kernel_spmd

    nc = _build()

    wq_b = np.ascontiguousarray(Wq.astype(BF))
    wk_b = np.ascontiguousarray(Wk.astype(BF))
    wv_b = np.ascontiguousarray(Wv.astype(BF))
    wo_b = np.ascontiguousarray(Wo.astype(BF))
    bq_f = np.ascontiguousarray(bq.astype(np.float32))
    bk_f = np.ascontiguousarray(bk.astype(np.float32))
    bv_f = np.ascontiguousarray(bv.astype(np.float32))
    bo_f = np.ascontiguousarray(bo.astype(np.float32))

    in_maps = []
    for c in range(N_CORES):
        b, j = c // 2, c % 2
        xt_b = np.ascontiguousarray(hidden_states[b].T.astype(BF))
        in_maps.append({
            "xt": xt_b,
            "xqt": np.ascontiguousarray(xt_b[:, j * SQ:(j + 1) * SQ]),
            "wq": wq_b, "wk": wk_b, "wv": wv_b, "wo": wo_b,
            "bq": bq_f, "bk": bk_f, "bv": bv_f, "bo": bo_f,
            "mk": np.ascontiguousarray(
                attention_mask[b, 0, 0, :].astype(np.float32) + NEG_C),
        })

    res = run_bass_kernel_spmd(nc, in_maps, list(range(N_CORES)))

    full = np.empty((B, S, D), dtype=np.float32)
    for c in range(N_CORES):
        b, j = c // 2, c % 2
        full[b, j * SQ:(j + 1) * SQ, :] = res.results[c]["out"]
    return full
